# revision 2
# baseline (speedup 1.0000x reference)
"""Trainium2 Bass kernel for a MoE transformer decoder layer (B=1, S=1024,
H=2048; 32 q heads / 8 kv heads, head dim 64; 8 experts top-2, FFN 4096),
SPMD across 8 NeuronCores.

Sharding: attention is tensor-parallel over heads (4 q heads + 1 kv head per
core), the MoE is expert-parallel (1 expert per core, dense over all tokens;
zero gates contribute zero), combined with AllReduce collectives. On-chip
activations are stored transposed: [features(partitions), tokens(free)].

Precision: everything upstream of the router runs plain-f32 matmuls (the
top-2 routing decision is sensitive to upstream noise); the expert FFN runs
bf16; sum-of-squares reductions run f32r.
"""
import sys

for _p in ("/opt/trn_rl_repo", "/opt/pypackages"):
    if _p not in sys.path:
        sys.path.append(_p)

import numpy as np

import concourse.bass as bass
import concourse.mybir as mybir
import concourse.tile as tile

dt = mybir.dt
AF = mybir.ActivationFunctionType
ALU = mybir.AluOpType
P = 128


def build(nc, S, H, Dh, NQH, F, n_cores=8, eps=1e-5, mult=0.125, cap=30.0):
    HK = H // P          # K-tiles over H
    SK = S // P          # s-blocks
    FK = F // P          # m-tiles over F
    DCOL = NQH * Dh      # per-core q projection width
    MQ = DCOL // P       # q M-tiles (2 heads per tile when Dh=64)
    DK = max(1, DCOL // P)
    E = 8
    NC = min(512, S)
    SC = S // NC
    HPT = P // Dh        # heads per q M-tile

    f32, bf16, f32r = dt.float32, dt.bfloat16, dt.float32r

    def p_in(name, shape, d=f32):
        return nc.declare_dram_parameter(name, list(shape), d, isOutput=False)

    hsT = p_in("hsT", [H, S])
    wq = p_in("wq", [P, HK, DCOL])
    wk = p_in("wk", [P, HK, Dh])
    wv = p_in("wv", [P, HK, Dh])
    wo = p_in("wo", [P, DK, H])
    router = p_in("router", [P, HK, E])
    n2 = p_in("n2", [P, HK])
    n4 = p_in("n4", [P, HK])
    cos2 = p_in("cos2", [P, S])
    sin2 = p_in("sin2", [P, S])
    mtril = p_in("mtril", [P, P])
    madd = p_in("madd", [P, P])
    ident = p_in("ident", [P, P])
    onehot = p_in("onehot", [E, 1])
    w1 = p_in("w1", [FK, P, HK * P], bf16)
    wev = p_in("wev", [FK, P, HK * P], bf16)
    wed = p_in("wed", [HK, P, FK * P], bf16)

    outT = nc.declare_dram_parameter("outT", [H, S], f32, isOutput=True)
    rl_out = nc.declare_dram_parameter("rl", [E, S], f32, isOutput=True)

    ar1_in = nc.dram_tensor("ar1_in", [H, S], f32)
    ar1_out = nc.dram_tensor("ar1_out", [H, S], f32, addr_space="Shared")
    ar2_in = nc.dram_tensor("ar2_in", [H, S], f32)
    ar2_out = nc.dram_tensor("ar2_out", [H, S], f32, addr_space="Shared")
    h1_spill = nc.dram_tensor("h1_spill", [H, S], f32)

    rg = [list(range(n_cores))]

    with tile.TileContext(nc) as tc:
        const = tc.alloc_tile_pool(name="const", bufs=1)
        cos_sb = const.tile([P, S], f32)
        sin_sb = const.tile([P, S], f32)
        mtril_sb = const.tile([P, P], f32)
        madd_sb = const.tile([P, P], f32)
        ident_sb = const.tile([P, P], f32)
        onehot_sb = const.tile([E, 1], f32)
        n2_sb = const.tile([P, HK], f32)
        n4_sb = const.tile([P, HK], f32)
        ones_kr = const.tile([P, 1], f32r)
        ones_kf = const.tile([P, 1], f32)
        ones_m = const.tile([1, P], f32)
        nc.sync.dma_start(cos_sb[:], cos2[:])
        nc.sync.dma_start(sin_sb[:], sin2[:])
        nc.sync.dma_start(mtril_sb[:], mtril[:])
        nc.sync.dma_start(madd_sb[:], madd[:])
        nc.sync.dma_start(ident_sb[:], ident[:])
        nc.sync.dma_start(onehot_sb[:], onehot[:])
        nc.sync.dma_start(n2_sb[:], n2[:])
        nc.sync.dma_start(n4_sb[:], n4[:])
        nc.vector.memset(ones_kf[:], 1.0)
        nc.vector.tensor_copy(ones_kr[:], ones_kf[:])
        nc.vector.memset(ones_m[:], 1.0)

        rows = tc.alloc_tile_pool(name="rows", bufs=1)

        def sumsq_accum(ss_chunks, sq_f32r_tile, k, klast):
            for c in range(SC):
                nc.tensor.matmul(ss_chunks[c][:], ones_kr[:],
                                 sq_f32r_tile[:, c * NC:(c + 1) * NC],
                                 start=(k == 0), stop=(k == klast))

        def rowstats(ss_chunks, name):
            srow = rows.tile([1, S], f32, tag=f"srow_{name}", name=f"srow_{name}")
            for c in range(SC):
                nc.scalar.activation(srow[:, c * NC:(c + 1) * NC],
                                     ss_chunks[c][:], AF.Sqrt,
                                     bias=eps, scale=1.0 / H)
            rstd = rows.tile([1, S], f32, tag=f"rstd_{name}", name=f"rstd_{name}")
            nc.vector.reciprocal(rstd[:], srow[:])
            return rstd

        def bcast_row(row, pool, psum_pool, name):
            bc = pool.tile([P, S], f32, tag=f"bc_{name}", name=f"bc_{name}")
            for c in range(SC):
                pt = psum_pool.tile([P, NC], f32, tag="bc_ps", name="bc_ps")
                nc.tensor.matmul(pt[:], ones_m[:],
                                 row[:, c * NC:(c + 1) * NC],
                                 start=True, stop=True)
                nc.vector.tensor_copy(bc[:, c * NC:(c + 1) * NC], pt[:])
            return bc

        # =========== Phases A-D inside attention scope ===========
        with tc.tile_pool(name="attn", bufs=1) as attn:
            qrT = attn.tile([P, MQ, S], f32)
            krT = attn.tile([Dh, S], f32)
            vN = attn.tile([P, SK, Dh], f32)
            attnT = attn.tile([P, DK, S], f32)

            with tc.tile_pool(name="bigA", bufs=1) as bigA:
                x1T = bigA.tile([P, HK, S], f32)
                # ---- Phase A: load + norm1 (n1 folded into wq/wk/wv) ----
                with tc.tile_pool(name="phA", bufs=3) as phA, \
                     tc.tile_pool(name="psA", bufs=1, space="PSUM") as psA, \
                     tc.tile_pool(name="psBC", bufs=1, space="PSUM") as psBC:
                    ss1 = [psA.tile([1, NC], f32, tag=f"ss1_{c}", name=f"ss1_{c}")
                           for c in range(SK and SC)]
                    for k in range(HK):
                        nc.sync.dma_start(x1T[:, k, :], hsT[k * P:(k + 1) * P, :])
                        sq = phA.tile([P, S], f32r, tag="sq", name="sq")
                        nc.vector.tensor_tensor(sq[:], x1T[:, k, :], x1T[:, k, :],
                                                ALU.mult)
                        sumsq_accum(ss1, sq, k, HK - 1)
                    rstd1 = rowstats(ss1, "n1")
                    bc1 = bcast_row(rstd1, bigA, psBC, "n1")
                    for k in range(HK):
                        nc.vector.tensor_tensor(x1T[:, k, :], x1T[:, k, :],
                                                bc1[:], ALU.mult)

                # ---- Phase B: qkv + rope ----
                with tc.tile_pool(name="phB", bufs=2) as phB, \
                     tc.tile_pool(name="wts", bufs=1) as wts, \
                     tc.tile_pool(name="psB", bufs=2, space="PSUM") as psB:
                    wq_sb = wts.tile([P, HK, DCOL], f32)
                    nc.sync.dma_start(wq_sb[:], wq[:])
                    wk_sb = wts.tile([P, HK, Dh], f32)
                    nc.sync.dma_start(wk_sb[:], wk[:])
                    wv_sb = wts.tile([P, HK, Dh], f32)
                    nc.sync.dma_start(wv_sb[:], wv[:])

                    def rope_apply(dst, raw_sb, nh):
                        rot = phB.tile([P, S], f32, tag="rot", name="rot")
                        half = Dh // 2
                        for hh in range(nh):
                            base = hh * Dh
                            nc.scalar.mul(rot[base:base + half, :],
                                          raw_sb[base + half:base + Dh, :], -1.0)
                            nc.scalar.copy(rot[base + half:base + Dh, :],
                                           raw_sb[base:base + half, :])
                        rr = nh * Dh
                        t1 = phB.tile([P, S], f32, tag="ropet1", name="ropet1")
                        nc.vector.tensor_tensor(t1[:rr, :], raw_sb[:rr, :],
                                                cos_sb[:rr, :], ALU.mult)
                        nc.vector.tensor_tensor(rot[:rr, :], rot[:rr, :],
                                                sin_sb[:rr, :], ALU.mult)
                        nc.vector.tensor_tensor(dst, t1[:rr, :], rot[:rr, :],
                                                ALU.add)

                    for m in range(MQ):
                        qraw = phB.tile([P, S], f32, tag="qraw", name="qraw")
                        for c in range(SC):
                            pq = psB.tile([P, NC], f32, tag="pq", name="pq")
                            for k in range(HK):
                                nc.tensor.matmul(
                                    pq[:], wq_sb[:, k, m * P:(m + 1) * P],
                                    x1T[:, k, c * NC:(c + 1) * NC],
                                    start=(k == 0), stop=(k == HK - 1))
                            nc.vector.tensor_copy(qraw[:, c * NC:(c + 1) * NC],
                                                  pq[:])
                        rope_apply(qrT[:, m, :], qraw, HPT)
                    kraw = phB.tile([P, S], f32, tag="qraw", name="kraw")
                    for c in range(SC):
                        pk = psB.tile([Dh, NC], f32, tag="pk", name="pk")
                        for k in range(HK):
                            nc.tensor.matmul(pk[:], wk_sb[:, k, :],
                                             x1T[:, k, c * NC:(c + 1) * NC],
                                             start=(k == 0), stop=(k == HK - 1))
                        nc.vector.tensor_copy(kraw[:Dh, c * NC:(c + 1) * NC], pk[:])
                    rope_apply(krT[:], kraw, 1)
                    for sb in range(SK):
                        pv = psB.tile([P, Dh], f32, tag="pv", name="pv")
                        for k in range(HK):
                            nc.tensor.matmul(
                                pv[:], x1T[:, k, sb * P:(sb + 1) * P],
                                wv_sb[:, k, :],
                                start=(k == 0), stop=(k == HK - 1))
                        nc.vector.tensor_copy(vN[:, sb, :], pv[:])

            # ---- Phase C: attention (causal, softcap) ----
            with tc.tile_pool(name="phC", bufs=3) as phC, \
                 tc.tile_pool(name="psC", bufs=2, space="PSUM") as psC, \
                 tc.tile_pool(name="psCt", bufs=2, space="PSUM") as psCt, \
                 tc.tile_pool(name="psCa", bufs=2, space="PSUM") as psCa:
                for h in range(NQH):
                    qtile, qoff = h // HPT, (h % HPT) * Dh
                    for b in range(SK):
                        ncols = (b + 1) * P
                        pl = psC.tile([P, min(S, 1024)], f32, tag="pl", name="pl")
                        nch = (ncols + NC - 1) // NC
                        for ci in range(nch):
                            off = ci * NC
                            w = min(NC, ncols - off)
                            nc.tensor.matmul(
                                pl[:, off:off + w],
                                qrT[qoff:qoff + Dh, qtile, b * P:(b + 1) * P],
                                krT[:, off:off + w],
                                start=True, stop=True)
                        st = phC.tile([P, S], f32, tag="st", name="st")
                        nc.scalar.activation(st[:, :ncols], pl[:, :ncols],
                                             AF.Tanh, scale=mult / cap)
                        dg = st[:, b * P:ncols]
                        nc.vector.tensor_tensor(dg, dg, mtril_sb[:], ALU.mult)
                        nc.vector.tensor_tensor(dg, dg, madd_sb[:], ALU.add)
                        et = phC.tile([P, S], f32, tag="et", name="et")
                        den = phC.tile([P, 1], f32, tag="den", name="den")
                        nc.scalar.activation(et[:, :ncols], st[:, :ncols], AF.Exp,
                                             bias=-cap, scale=cap,
                                             accum_out=den[:])
                        rden = phC.tile([P, 1], f32, tag="rden", name="rden")
                        nc.vector.reciprocal(rden[:], den[:])
                        pr = phC.tile([P, S], f32, tag="pr", name="pr")
                        nc.vector.tensor_scalar_mul(pr[:, :ncols], et[:, :ncols],
                                                    rden[:])
                        pa = psCa.tile([Dh, P], f32, tag="pa", name="pa")
                        for tb in range(b + 1):
                            ptr = psCt.tile([P, P], f32, tag="ptr", name="ptr")
                            nc.tensor.transpose(ptr[:],
                                                pr[:, tb * P:(tb + 1) * P],
                                                ident_sb[:])
                            prT = phC.tile([P, P], f32, tag="prT", name="prT")
                            nc.vector.tensor_copy(prT[:], ptr[:])
                            nc.tensor.matmul(pa[:], vN[:, tb, :], prT[:],
                                             start=(tb == 0), stop=(tb == b))
                        nc.vector.tensor_copy(
                            attnT[qoff:qoff + Dh, qtile, b * P:(b + 1) * P],
                            pa[:])

            # ---- Phase D: wo partial ----
            with tc.tile_pool(name="phD", bufs=2) as phD, \
                 tc.tile_pool(name="wod", bufs=1) as wod, \
                 tc.tile_pool(name="psD", bufs=2, space="PSUM") as psD:
                wo_sb = wod.tile([P, DK, H], f32)
                nc.sync.dma_start(wo_sb[:], wo[:])
                for m in range(HK):
                    stage = phD.tile([P, S], f32, tag="dstage", name="dstage")
                    for c in range(SC):
                        po = psD.tile([P, NC], f32, tag="po", name="po")
                        for kd in range(DK):
                            nc.tensor.matmul(
                                po[:], wo_sb[:, kd, m * P:(m + 1) * P],
                                attnT[:, kd, c * NC:(c + 1) * NC],
                                start=(kd == 0), stop=(kd == DK - 1))
                        nc.vector.tensor_copy(stage[:, c * NC:(c + 1) * NC],
                                              po[:])
                    nc.sync.dma_start(ar1_in[m * P:(m + 1) * P, :], stage[:])

        nc.gpsimd.collective_compute(
            "AllReduce", ALU.add, replica_groups=rg,
            ins=[ar1_in[:]], outs=[ar1_out[:]])

        # =========== Phases E-G scope ===========
        with tc.tile_pool(name="bigE", bufs=1) as bigE, \
             tc.tile_pool(name="gates", bufs=1) as gates:
            x2bf = bigE.tile([P, HK, S], bf16)
            gate_bc = gates.tile([P, S], f32)
            with tc.tile_pool(name="psR", bufs=1, space="PSUM") as psR:
                rl_ps = [psR.tile([E, NC], f32, tag=f"rl_{c}", name=f"rl_{c}")
                         for c in range(SC)]
                # ---- Phase E: h1, x2, router ----
                with tc.tile_pool(name="bigH1", bufs=1) as bigH1, \
                     tc.tile_pool(name="phE", bufs=3) as phE, \
                     tc.tile_pool(name="psE", bufs=1, space="PSUM") as psE, \
                     tc.tile_pool(name="psBC2", bufs=1, space="PSUM") as psBC2:
                    h1T = bigH1.tile([P, HK, S], f32)
                    ss2 = [psE.tile([1, NC], f32, tag=f"ss2_{c}", name=f"ss2_{c}")
                           for c in range(SC)]
                    for k in range(HK):
                        nc.sync.dma_start(h1T[:, k, :],
                                          ar1_out[k * P:(k + 1) * P, :])
                        sq = phE.tile([P, S], f32r, tag="sqe", name="sqe")
                        nc.vector.tensor_tensor(sq[:], h1T[:, k, :], h1T[:, k, :],
                                                ALU.mult)
                        sumsq_accum(ss2, sq, k, HK - 1)
                    rstd2 = rowstats(ss2, "n2")
                    bc2 = bcast_row(rstd2, bigE, psBC2, "n2")
                    ss3 = [psE.tile([1, NC], f32, tag=f"ss3_{c}", name=f"ss3_{c}")
                           for c in range(SC)]
                    for k in range(HK):
                        nc.vector.tensor_tensor(h1T[:, k, :], h1T[:, k, :],
                                                bc2[:], ALU.mult)
                        nc.vector.tensor_scalar_mul(h1T[:, k, :], h1T[:, k, :],
                                                    n2_sb[:, k:k + 1])
                        hs2 = phE.tile([P, S], f32, tag="hs2", name="hs2")
                        nc.sync.dma_start(hs2[:], hsT[k * P:(k + 1) * P, :])
                        nc.vector.tensor_tensor(h1T[:, k, :], h1T[:, k, :],
                                                hs2[:], ALU.add)
                        nc.sync.dma_start(h1_spill[k * P:(k + 1) * P, :],
                                          h1T[:, k, :])
                        sq = phE.tile([P, S], f32r, tag="sqe", name="sqe2")
                        nc.vector.tensor_tensor(sq[:], h1T[:, k, :], h1T[:, k, :],
                                                ALU.mult)
                        sumsq_accum(ss3, sq, k, HK - 1)
                    rstd3 = rowstats(ss3, "n3")
                    bc3 = bcast_row(rstd3, bigE, psBC2, "n3")
                    rt_sb = phE.tile([P, HK, E], f32, tag="rt", name="rt")
                    nc.sync.dma_start(rt_sb[:], router[:])
                    for k in range(HK):
                        x2f = phE.tile([P, S], f32, tag="x2f", name="x2f")
                        nc.vector.tensor_tensor(x2f[:], h1T[:, k, :], bc3[:],
                                                ALU.mult)
                        nc.vector.tensor_copy(x2bf[:, k, :], x2f[:])
                        for c in range(SC):
                            nc.tensor.matmul(rl_ps[c][:], rt_sb[:, k, :],
                                             x2f[:, c * NC:(c + 1) * NC],
                                             start=(k == 0), stop=(k == HK - 1))

                # ---- Phase F: gates ----
                with tc.tile_pool(name="phF", bufs=1) as phF, \
                     tc.tile_pool(name="psF", bufs=1, space="PSUM") as psF:
                    rl_sb = phF.tile([E, S], f32, tag="rl", name="rl_sb")
                    for c in range(SC):
                        nc.vector.tensor_copy(rl_sb[:, c * NC:(c + 1) * NC],
                                              rl_ps[c][:])
                    nc.sync.dma_start(rl_out[:], rl_sb[:])
                    t4 = phF.tile([4, S], f32, tag="t4", name="t4")
                    t2 = phF.tile([2, S], f32, tag="t2", name="t2")
                    mrow = phF.tile([1, S], f32, tag="mrow", name="mrow")
                    nc.vector.tensor_tensor(t4[:], rl_sb[0:4, :], rl_sb[4:8, :],
                                            ALU.max)
                    nc.vector.tensor_tensor(t2[:], t4[0:2, :], t4[2:4, :], ALU.max)
                    nc.vector.tensor_tensor(mrow[:], t2[0:1, :], t2[1:2, :],
                                            ALU.max)
                    m8 = phF.tile([E, S], f32, tag="m8", name="m8")
                    for c in range(SC):
                        pf = psF.tile([E, NC], f32, tag="pf", name="pf")
                        nc.tensor.matmul(pf[:], ones_m[:, :E],
                                         mrow[:, c * NC:(c + 1) * NC],
                                         start=True, stop=True)
                        nc.vector.tensor_copy(m8[:, c * NC:(c + 1) * NC], pf[:])
                    eu = phF.tile([E, S], f32, tag="eu", name="eu")
                    nc.vector.tensor_tensor(eu[:], rl_sb[:], m8[:], ALU.subtract)
                    eux = phF.tile([E, S], f32, tag="eux", name="eux")
                    nc.scalar.activation(eux[:], eu[:], AF.Exp)
                    m1 = phF.tile([1, S], f32, tag="m1", name="m1")
                    nc.vector.tensor_tensor(t4[:], eux[0:4, :], eux[4:8, :],
                                            ALU.max)
                    nc.vector.tensor_tensor(t2[:], t4[0:2, :], t4[2:4, :], ALU.max)
                    nc.vector.tensor_tensor(m1[:], t2[0:1, :], t2[1:2, :], ALU.max)
                    m1_8 = phF.tile([E, S], f32, tag="m1_8", name="m1_8")
                    for c in range(SC):
                        pf = psF.tile([E, NC], f32, tag="pf", name="pf2")
                        nc.tensor.matmul(pf[:], ones_m[:, :E],
                                         m1[:, c * NC:(c + 1) * NC],
                                         start=True, stop=True)
                        nc.vector.tensor_copy(m1_8[:, c * NC:(c + 1) * NC], pf[:])
                    eq1 = phF.tile([E, S], f32, tag="eq1", name="eq1")
                    nc.vector.tensor_tensor(eq1[:], eux[:], m1_8[:], ALU.is_equal)
                    nc.vector.tensor_scalar(eq1[:], eq1[:], -1.0, 1.0,
                                            ALU.mult, ALU.add)
                    eum = phF.tile([E, S], f32, tag="eum", name="eum")
                    nc.vector.tensor_tensor(eum[:], eux[:], eq1[:], ALU.mult)
                    m2 = phF.tile([1, S], f32, tag="m2", name="m2")
                    nc.vector.tensor_tensor(t4[:], eum[0:4, :], eum[4:8, :],
                                            ALU.max)
                    nc.vector.tensor_tensor(t2[:], t4[0:2, :], t4[2:4, :], ALU.max)
                    nc.vector.tensor_tensor(m2[:], t2[0:1, :], t2[1:2, :], ALU.max)
                    ei = phF.tile([1, S], f32, tag="ei", name="ei")
                    for c in range(SC):
                        pf1 = psF.tile([1, NC], f32, tag="pf1", name="pf1")
                        nc.tensor.matmul(pf1[:], onehot_sb[:],
                                         eux[:, c * NC:(c + 1) * NC],
                                         start=True, stop=True)
                        nc.vector.tensor_copy(ei[:, c * NC:(c + 1) * NC], pf1[:])
                    e1 = phF.tile([1, S], f32, tag="e1", name="e1")
                    e2 = phF.tile([1, S], f32, tag="e2", name="e2")
                    nc.vector.tensor_tensor(e1[:], ei[:], m1[:], ALU.is_equal)
                    nc.vector.tensor_tensor(e2[:], ei[:], m2[:], ALU.is_equal)
                    nc.vector.tensor_tensor(e1[:], e1[:], e2[:], ALU.max)
                    den = phF.tile([1, S], f32, tag="denF", name="denF")
                    nc.vector.tensor_tensor(den[:], m1[:], m2[:], ALU.add)
                    rden = phF.tile([1, S], f32, tag="rdenF", name="rdenF")
                    nc.vector.reciprocal(rden[:], den[:])
                    gate = phF.tile([1, S], f32, tag="gate", name="gate")
                    nc.vector.tensor_tensor(gate[:], ei[:], e1[:], ALU.mult)
                    nc.vector.tensor_tensor(gate[:], gate[:], rden[:], ALU.mult)
                    for c in range(SC):
                        pf2 = psF.tile([P, NC], f32, tag="pf2", name="pfg")
                        nc.tensor.matmul(pf2[:], ones_m[:],
                                         gate[:, c * NC:(c + 1) * NC],
                                         start=True, stop=True)
                        nc.vector.tensor_copy(gate_bc[:, c * NC:(c + 1) * NC],
                                              pf2[:])

            # ---- Phase G: expert FFN (bf16) ----
            with tc.tile_pool(name="bigG", bufs=1) as bigG, \
                 tc.tile_pool(name="phG", bufs=2) as phG, \
                 tc.tile_pool(name="psG1", bufs=2, space="PSUM") as psG1, \
                 tc.tile_pool(name="psG2", bufs=2, space="PSUM") as psG2:
                hT = bigG.tile([P, FK, S], bf16)
                for mf in range(FK):
                    w1t = phG.tile([P, HK * P], bf16, tag="w1t", name="w1t")
                    nc.sync.dma_start(w1t[:], w1[mf])
                    vt = phG.tile([P, HK * P], bf16, tag="vt", name="vt")
                    nc.sync.dma_start(vt[:], wev[mf])
                    for c in range(SC):
                        pw = psG1.tile([P, NC], f32, tag="pw", name="pw")
                        pv = psG1.tile([P, NC], f32, tag="pvm", name="pvm")
                        for k in range(HK):
                            nc.tensor.matmul(pw[:], w1t[:, k * P:(k + 1) * P],
                                             x2bf[:, k, c * NC:(c + 1) * NC],
                                             start=(k == 0), stop=(k == HK - 1))
                        for k in range(HK):
                            nc.tensor.matmul(pv[:], vt[:, k * P:(k + 1) * P],
                                             x2bf[:, k, c * NC:(c + 1) * NC],
                                             start=(k == 0), stop=(k == HK - 1))
                        gl = phG.tile([P, NC], bf16, tag="gl", name="gl")
                        nc.scalar.activation(gl[:], pw[:], AF.Gelu)
                        nc.vector.tensor_tensor(hT[:, mf, c * NC:(c + 1) * NC],
                                                gl[:], pv[:], ALU.mult)
                for mo in range(HK):
                    dt_sb = phG.tile([P, FK * P], bf16, tag="dt", name="dt_sb")
                    nc.sync.dma_start(dt_sb[:], wed[mo])
                    stage = phG.tile([P, S], f32, tag="gstage", name="gstage")
                    for c in range(SC):
                        pd = psG2.tile([P, NC], f32, tag="pd", name="pd")
                        for k in range(FK):
                            nc.tensor.matmul(pd[:], dt_sb[:, k * P:(k + 1) * P],
                                             hT[:, k, c * NC:(c + 1) * NC],
                                             start=(k == 0), stop=(k == FK - 1))
                        nc.vector.tensor_tensor(
                            stage[:, c * NC:(c + 1) * NC], pd[:],
                            gate_bc[:, c * NC:(c + 1) * NC], ALU.mult)
                    nc.sync.dma_start(ar2_in[mo * P:(mo + 1) * P, :], stage[:])

        nc.gpsimd.collective_compute(
            "AllReduce", ALU.add, replica_groups=rg,
            ins=[ar2_in[:]], outs=[ar2_out[:]])

        # ---- Phase H: final norm + residual ----
        with tc.tile_pool(name="bigM", bufs=1) as bigM, \
             tc.tile_pool(name="phH", bufs=3) as phH, \
             tc.tile_pool(name="psH", bufs=1, space="PSUM") as psH, \
             tc.tile_pool(name="psBC4", bufs=1, space="PSUM") as psBC4:
            moT = bigM.tile([P, HK, S], f32)
            ss4 = [psH.tile([1, NC], f32, tag=f"ss4_{c}", name=f"ss4_{c}")
                   for c in range(SC)]
            for k in range(HK):
                nc.sync.dma_start(moT[:, k, :], ar2_out[k * P:(k + 1) * P, :])
                sq = phH.tile([P, S], f32r, tag="sqh", name="sqh")
                nc.vector.tensor_tensor(sq[:], moT[:, k, :], moT[:, k, :],
                                        ALU.mult)
                sumsq_accum(ss4, sq, k, HK - 1)
            rstd4 = rowstats(ss4, "n4")
            bc4 = bcast_row(rstd4, bigM, psBC4, "n4")
            for k in range(HK):
                h1r = phH.tile([P, S], f32, tag="h1r", name="h1r")
                nc.sync.dma_start(h1r[:], h1_spill[k * P:(k + 1) * P, :])
                nc.vector.tensor_tensor(moT[:, k, :], moT[:, k, :], bc4[:],
                                        ALU.mult)
                nc.vector.tensor_scalar_mul(moT[:, k, :], moT[:, k, :],
                                            n4_sb[:, k:k + 1])
                ostage = phH.tile([P, S], f32, tag="ostage", name="ostage")
                nc.vector.tensor_tensor(ostage[:], moT[:, k, :], h1r[:], ALU.add)
                nc.sync.dma_start(outT[k * P:(k + 1) * P, :], ostage[:])

        const.release()
        rows.release()
    return nc


def host_prep(inputs_np, core, S, H, Dh, NQH, F):
    """Build the per-core in_map from full inputs (numpy)."""
    import numpy as np
    import ml_dtypes
    HK, FK = H // P, F // P
    DCOL = NQH * Dh
    DK = max(1, DCOL // P)
    E = 8
    f32 = np.float32
    i = core
    hs = np.ascontiguousarray(inputs_np["hidden_states"][0].T).astype(f32)
    n1 = inputs_np["n1"].astype(f32)
    n3 = inputs_np["n3"].astype(f32)
    wq_f = (n1[:, None] * inputs_np["wq"])[:, DCOL * i:DCOL * (i + 1)]
    wk_f = (n1[:, None] * inputs_np["wk"])[:, Dh * i:Dh * (i + 1)]
    wv_f = (n1[:, None] * inputs_np["wv"])[:, Dh * i:Dh * (i + 1)]
    wo_i = inputs_np["wo"][DCOL * i:DCOL * (i + 1), :]
    router_f = n3[:, None] * inputs_np["router"]
    w1_f = (n3[:, None] * inputs_np["we_w1"][i]).astype(ml_dtypes.bfloat16)
    wv_e = (n3[:, None] * inputs_np["we_v"][i]).astype(ml_dtypes.bfloat16)
    wd_e = inputs_np["we_d"][i].astype(ml_dtypes.bfloat16)

    def ktile(w, width):  # [H, width] -> [128, HK, width]
        return np.ascontiguousarray(
            w.reshape(HK, P, width).transpose(1, 0, 2)).astype(f32)

    inv = 1.0 / (10000.0 ** (np.arange(0, Dh, 2) / Dh))
    t = np.arange(S, dtype=np.float64)
    ph = t[:, None] * inv[None, :]
    ph = np.concatenate([ph, ph], -1)
    cosT = np.cos(ph).T.astype(f32)
    sinT = np.sin(ph).T.astype(f32)
    reps = P // Dh
    tril = np.tril(np.ones((P, P), f32))
    onehot = np.zeros((E, 1), f32)
    onehot[i, 0] = 1.0
    return {
        "hsT": hs,
        "wq": ktile(wq_f, DCOL),
        "wk": ktile(wk_f, Dh),
        "wv": ktile(wv_f, Dh),
        "wo": np.ascontiguousarray(
            wo_i.reshape(DK, P, H).transpose(1, 0, 2)).astype(f32),
        "router": ktile(router_f, E),
        "n2": np.ascontiguousarray(
            inputs_np["n2"].astype(f32).reshape(HK, P).T),
        "n4": np.ascontiguousarray(
            inputs_np["n4"].astype(f32).reshape(HK, P).T),
        "cos2": np.tile(cosT, (reps, 1)),
        "sin2": np.tile(sinT, (reps, 1)),
        "mtril": tril,
        "madd": np.where(tril > 0, 0.0, -50.0).astype(f32),
        "ident": np.eye(P, dtype=f32),
        "onehot": onehot,
        "w1": np.ascontiguousarray(
            w1_f.reshape(HK, P, FK, P).transpose(2, 1, 0, 3).reshape(FK, P, HK * P)),
        "wev": np.ascontiguousarray(
            wv_e.reshape(HK, P, FK, P).transpose(2, 1, 0, 3).reshape(FK, P, HK * P)),
        "wed": np.ascontiguousarray(
            wd_e.reshape(FK, P, HK, P).transpose(2, 1, 0, 3).reshape(HK, P, FK * P)),
    }


_CACHE = {}

S, H, Dh, NQH, F, NCORES = 1024, 2048, 64, 4, 4096, 8


def _get_nc():
    if "nc" not in _CACHE:
        from concourse import bacc
        nc = bacc.Bacc("TRN2", target_bir_lowering=False, debug=False,
                       num_devices=NCORES)
        build(nc, S, H, Dh, NQH, F, n_cores=NCORES)
        nc.compile()
        _CACHE["nc"] = nc
    return _CACHE["nc"]


def kernel(**inputs):
    from concourse.bass_utils import run_bass_kernel_spmd
    nc = _get_nc()
    inputs = {k: np.asarray(v) for k, v in inputs.items()}
    in_maps = [host_prep(inputs, i, S, H, Dh, NQH, F) for i in range(NCORES)]
    res = run_bass_kernel_spmd(nc, in_maps, list(range(NCORES))).results
    out = np.ascontiguousarray(res[0]["outT"].T)[None].astype(np.float32)
    rl = np.ascontiguousarray(res[0]["rl"].T).astype(np.float32)
    return out, rl


# revision 3
# speedup vs baseline: 1.1732x; 1.1732x over previous
"""Trainium2 Bass kernel for a MoE transformer decoder layer (B=1, S=1024,
H=2048; 32 q heads / 8 kv heads, head dim 64; 8 experts top-2, FFN 4096),
SPMD across 8 NeuronCores.

Sharding: attention is tensor-parallel over heads (4 q heads + 1 kv head per
core), the MoE is expert-parallel (1 expert per core, dense over all tokens;
zero gates contribute zero), combined with AllReduce collectives. On-chip
activations are stored transposed: [features(partitions), tokens(free)].

Precision: everything upstream of the router runs plain-f32 matmuls (the
top-2 routing decision is sensitive to upstream noise); the expert FFN runs
bf16; sum-of-squares reductions run f32r.
"""
import sys

for _p in ("/opt/trn_rl_repo", "/opt/pypackages"):
    if _p not in sys.path:
        sys.path.append(_p)

import numpy as np

import concourse.bass as bass
import concourse.mybir as mybir
import concourse.tile as tile

dt = mybir.dt
AF = mybir.ActivationFunctionType
ALU = mybir.AluOpType
P = 128


def build(nc, S, H, Dh, NQH, F, n_cores=8, eps=1e-5, mult=0.125, cap=30.0):
    HK = H // P          # K-tiles over H
    SK = S // P          # s-blocks
    FK = F // P          # m-tiles over F
    DCOL = NQH * Dh      # per-core q projection width
    MQ = DCOL // P       # q M-tiles (2 heads per tile when Dh=64)
    DK = max(1, DCOL // P)
    E = 8
    NC = min(512, S)
    SC = S // NC
    HPT = P // Dh        # heads per q M-tile

    f32, bf16, f32r = dt.float32, dt.bfloat16, dt.float32r

    def p_in(name, shape, d=f32):
        return nc.declare_dram_parameter(name, list(shape), d, isOutput=False)

    hsT = p_in("hsT", [H, S])
    wq = p_in("wq", [P, HK, DCOL])
    wk = p_in("wk", [P, HK, Dh])
    wv = p_in("wv", [P, HK, Dh])
    wo = p_in("wo", [P, DK, H])
    router = p_in("router", [P, HK, E])
    n2 = p_in("n2", [P, HK])
    n4 = p_in("n4", [P, HK])
    cos2 = p_in("cos2", [P, S])
    sin2 = p_in("sin2", [P, S])
    mtril = p_in("mtril", [P, P])
    madd = p_in("madd", [P, P])
    ident = p_in("ident", [P, P])
    onehot_r = p_in("onehot_r", [1, E])
    w1 = p_in("w1", [FK, P, HK * P], bf16)
    wev = p_in("wev", [FK, P, HK * P], bf16)
    wed = p_in("wed", [HK, P, FK * P], bf16)

    outT = nc.declare_dram_parameter("outT", [H, S], f32, isOutput=True)
    rl_out = nc.declare_dram_parameter("rl", [E, S], f32, isOutput=True)

    ar1_in = nc.dram_tensor("ar1_in", [H, S], f32)
    ar1_out = nc.dram_tensor("ar1_out", [H, S], f32, addr_space="Shared")
    ar2_in = nc.dram_tensor("ar2_in", [H, S], f32)
    ar2_out = nc.dram_tensor("ar2_out", [H, S], f32, addr_space="Shared")
    h1_spill = nc.dram_tensor("h1_spill", [H, S], f32)

    rg = [list(range(n_cores))]

    with tile.TileContext(nc) as tc:
        const = tc.alloc_tile_pool(name="const", bufs=1)
        cos_sb = const.tile([P, S], f32)
        sin_sb = const.tile([P, S], f32)
        mtril_sb = const.tile([P, P], f32)
        madd_sb = const.tile([P, P], f32)
        ident_sb = const.tile([P, P], f32)
        n2_sb = const.tile([P, HK], f32)
        n4_sb = const.tile([P, HK], f32)
        ones_kr = const.tile([P, 1], f32r)
        ones_kf = const.tile([P, 1], f32)
        ones_m = const.tile([1, P], f32)
        nc.sync.dma_start(cos_sb[:], cos2[:])
        nc.sync.dma_start(sin_sb[:], sin2[:])
        nc.sync.dma_start(mtril_sb[:], mtril[:])
        nc.sync.dma_start(madd_sb[:], madd[:])
        nc.sync.dma_start(ident_sb[:], ident[:])
        nc.sync.dma_start(n2_sb[:], n2[:])
        nc.sync.dma_start(n4_sb[:], n4[:])
        nc.vector.memset(ones_kf[:], 1.0)
        nc.vector.tensor_copy(ones_kr[:], ones_kf[:])
        nc.vector.memset(ones_m[:], 1.0)

        onehot_sbr = const.tile([1, E], f32)
        nc.sync.dma_start(onehot_sbr[:], onehot_r[:])

        rows = tc.alloc_tile_pool(name="rows", bufs=1)

        def sumsq_accum(ss_chunks, sq_f32r_tile, k, klast):
            for c in range(SC):
                nc.tensor.matmul(ss_chunks[c][:], ones_kr[:],
                                 sq_f32r_tile[:, c * NC:(c + 1) * NC],
                                 start=(k == 0), stop=(k == klast))

        def rowstats(ss_chunks, name):
            srow = rows.tile([1, S], f32, tag=f"srow_{name}", name=f"srow_{name}")
            for c in range(SC):
                nc.scalar.activation(srow[:, c * NC:(c + 1) * NC],
                                     ss_chunks[c][:], AF.Sqrt,
                                     bias=eps, scale=1.0 / H)
            rstd = rows.tile([1, S], f32, tag=f"rstd_{name}", name=f"rstd_{name}")
            nc.vector.reciprocal(rstd[:], srow[:])
            return rstd

        def bcast_row(row, pool, psum_pool, name):
            bc = pool.tile([P, S], f32, tag=f"bc_{name}", name=f"bc_{name}")
            for c in range(SC):
                pt = psum_pool.tile([P, NC], f32, tag="bc_ps", name="bc_ps")
                nc.tensor.matmul(pt[:], ones_m[:],
                                 row[:, c * NC:(c + 1) * NC],
                                 start=True, stop=True)
                nc.vector.tensor_copy(bc[:, c * NC:(c + 1) * NC], pt[:])
            return bc

        # =========== Phases A-D inside attention scope ===========
        with tc.tile_pool(name="attn", bufs=1) as attn:
            qrT = attn.tile([P, MQ, S], f32)
            krT = attn.tile([Dh, S], f32)
            vN = attn.tile([P, SK, Dh], f32)
            attnT = attn.tile([P, DK, S], f32)

            with tc.tile_pool(name="bigA", bufs=1) as bigA:
                x1T = bigA.tile([P, HK, S], f32)
                # ---- Phase A: load + norm1 (n1 folded into wq/wk/wv) ----
                with tc.tile_pool(name="phA", bufs=3) as phA, \
                     tc.tile_pool(name="psA", bufs=1, space="PSUM") as psA, \
                     tc.tile_pool(name="psBC", bufs=1, space="PSUM") as psBC:
                    ss1 = [psA.tile([1, NC], f32, tag=f"ss1_{c}", name=f"ss1_{c}")
                           for c in range(SK and SC)]
                    for k in range(HK):
                        nc.sync.dma_start(x1T[:, k, :], hsT[k * P:(k + 1) * P, :])
                        sq = phA.tile([P, S], f32r, tag="sq", name="sq")
                        nc.vector.tensor_tensor(sq[:], x1T[:, k, :], x1T[:, k, :],
                                                ALU.mult)
                        sumsq_accum(ss1, sq, k, HK - 1)
                    rstd1 = rowstats(ss1, "n1")
                    bc1 = bcast_row(rstd1, bigA, psBC, "n1")
                    for k in range(HK):
                        nc.vector.tensor_tensor(x1T[:, k, :], x1T[:, k, :],
                                                bc1[:], ALU.mult)

                # ---- Phase B: qkv + rope ----
                with tc.tile_pool(name="phB", bufs=2) as phB, \
                     tc.tile_pool(name="wts", bufs=1) as wts, \
                     tc.tile_pool(name="psB", bufs=2, space="PSUM") as psB:
                    wq_sb = wts.tile([P, HK, DCOL], f32)
                    nc.sync.dma_start(wq_sb[:], wq[:])
                    wk_sb = wts.tile([P, HK, Dh], f32)
                    nc.sync.dma_start(wk_sb[:], wk[:])
                    wv_sb = wts.tile([P, HK, Dh], f32)
                    nc.sync.dma_start(wv_sb[:], wv[:])

                    def rope_apply(dst, raw_sb, nh):
                        rot = phB.tile([P, S], f32, tag="rot", name="rot")
                        half = Dh // 2
                        for hh in range(nh):
                            base = hh * Dh
                            nc.scalar.mul(rot[base:base + half, :],
                                          raw_sb[base + half:base + Dh, :], -1.0)
                            nc.scalar.copy(rot[base + half:base + Dh, :],
                                           raw_sb[base:base + half, :])
                        rr = nh * Dh
                        t1 = phB.tile([P, S], f32, tag="ropet1", name="ropet1")
                        nc.vector.tensor_tensor(t1[:rr, :], raw_sb[:rr, :],
                                                cos_sb[:rr, :], ALU.mult)
                        nc.vector.tensor_tensor(rot[:rr, :], rot[:rr, :],
                                                sin_sb[:rr, :], ALU.mult)
                        nc.vector.tensor_tensor(dst, t1[:rr, :], rot[:rr, :],
                                                ALU.add)

                    for m in range(MQ):
                        qraw = phB.tile([P, S], f32, tag="qraw", name="qraw")
                        for c in range(SC):
                            pq = psB.tile([P, NC], f32, tag="pq", name="pq")
                            for k in range(HK):
                                nc.tensor.matmul(
                                    pq[:], wq_sb[:, k, m * P:(m + 1) * P],
                                    x1T[:, k, c * NC:(c + 1) * NC],
                                    start=(k == 0), stop=(k == HK - 1))
                            nc.vector.tensor_copy(qraw[:, c * NC:(c + 1) * NC],
                                                  pq[:])
                        rope_apply(qrT[:, m, :], qraw, HPT)
                    kraw = phB.tile([P, S], f32, tag="qraw", name="kraw")
                    for c in range(SC):
                        pk = psB.tile([Dh, NC], f32, tag="pk", name="pk")
                        for k in range(HK):
                            nc.tensor.matmul(pk[:], wk_sb[:, k, :],
                                             x1T[:, k, c * NC:(c + 1) * NC],
                                             start=(k == 0), stop=(k == HK - 1))
                        nc.vector.tensor_copy(kraw[:Dh, c * NC:(c + 1) * NC], pk[:])
                    rope_apply(krT[:], kraw, 1)
                    for sb in range(SK):
                        pv = psB.tile([P, Dh], f32, tag="pv", name="pv")
                        for k in range(HK):
                            nc.tensor.matmul(
                                pv[:], x1T[:, k, sb * P:(sb + 1) * P],
                                wv_sb[:, k, :],
                                start=(k == 0), stop=(k == HK - 1))
                        nc.vector.tensor_copy(vN[:, sb, :], pv[:])

            # ---- Phase C: attention (causal, softcap) ----
            with tc.tile_pool(name="phC", bufs=3) as phC, \
                 tc.tile_pool(name="psC", bufs=2, space="PSUM") as psC, \
                 tc.tile_pool(name="psCt", bufs=2, space="PSUM") as psCt, \
                 tc.tile_pool(name="psCa", bufs=2, space="PSUM") as psCa:
                for h in range(NQH):
                    qtile, qoff = h // HPT, (h % HPT) * Dh
                    for b in range(SK):
                        ncols = (b + 1) * P
                        pl = psC.tile([P, min(S, 1024)], f32, tag="pl", name="pl")
                        nch = (ncols + NC - 1) // NC
                        for ci in range(nch):
                            off = ci * NC
                            w = min(NC, ncols - off)
                            nc.tensor.matmul(
                                pl[:, off:off + w],
                                qrT[qoff:qoff + Dh, qtile, b * P:(b + 1) * P],
                                krT[:, off:off + w],
                                start=True, stop=True)
                        st = phC.tile([P, S], f32, tag="st", name="st")
                        nc.scalar.activation(st[:, :ncols], pl[:, :ncols],
                                             AF.Tanh, scale=mult / cap)
                        dg = st[:, b * P:ncols]
                        nc.vector.tensor_tensor(dg, dg, mtril_sb[:], ALU.mult)
                        nc.vector.tensor_tensor(dg, dg, madd_sb[:], ALU.add)
                        et = phC.tile([P, S], f32, tag="et", name="et")
                        den = phC.tile([P, 1], f32, tag="den", name="den")
                        nc.scalar.activation(et[:, :ncols], st[:, :ncols], AF.Exp,
                                             bias=-cap, scale=cap,
                                             accum_out=den[:])
                        rden = phC.tile([P, 1], f32, tag="rden", name="rden")
                        nc.vector.reciprocal(rden[:], den[:])
                        pr = phC.tile([P, S], f32, tag="pr", name="pr")
                        nc.vector.tensor_scalar_mul(pr[:, :ncols], et[:, :ncols],
                                                    rden[:])
                        pa = psCa.tile([Dh, P], f32, tag="pa", name="pa")
                        for tb in range(b + 1):
                            ptr = psCt.tile([P, P], f32, tag="ptr", name="ptr")
                            nc.tensor.transpose(ptr[:],
                                                pr[:, tb * P:(tb + 1) * P],
                                                ident_sb[:])
                            prT = phC.tile([P, P], f32, tag="prT", name="prT")
                            nc.vector.tensor_copy(prT[:], ptr[:])
                            nc.tensor.matmul(pa[:], vN[:, tb, :], prT[:],
                                             start=(tb == 0), stop=(tb == b))
                        nc.vector.tensor_copy(
                            attnT[qoff:qoff + Dh, qtile, b * P:(b + 1) * P],
                            pa[:])

            # ---- Phase D: wo partial ----
            with tc.tile_pool(name="phD", bufs=2) as phD, \
                 tc.tile_pool(name="wod", bufs=1) as wod, \
                 tc.tile_pool(name="psD", bufs=2, space="PSUM") as psD:
                wo_sb = wod.tile([P, DK, H], f32)
                nc.sync.dma_start(wo_sb[:], wo[:])
                for m in range(HK):
                    stage = phD.tile([P, S], f32, tag="dstage", name="dstage")
                    for c in range(SC):
                        po = psD.tile([P, NC], f32, tag="po", name="po")
                        for kd in range(DK):
                            nc.tensor.matmul(
                                po[:], wo_sb[:, kd, m * P:(m + 1) * P],
                                attnT[:, kd, c * NC:(c + 1) * NC],
                                start=(kd == 0), stop=(kd == DK - 1))
                        nc.vector.tensor_copy(stage[:, c * NC:(c + 1) * NC],
                                              po[:])
                    nc.sync.dma_start(ar1_in[m * P:(m + 1) * P, :], stage[:])

        _half = (HK // 2) * P
        nc.gpsimd.collective_compute(
            "AllReduce", ALU.add, replica_groups=rg,
            ins=[ar1_in[0:_half, :]], outs=[ar1_out[0:_half, :]])
        nc.gpsimd.collective_compute(
            "AllReduce", ALU.add, replica_groups=rg,
            ins=[ar1_in[_half:H, :]], outs=[ar1_out[_half:H, :]])

        # =========== Phases E-G scope ===========
        with tc.tile_pool(name="bigE", bufs=1) as bigE, \
             tc.tile_pool(name="gates", bufs=1) as gates:
            x2bf = bigE.tile([P, HK, S], bf16)
            gate_bc = gates.tile([P, S], f32)
            with tc.tile_pool(name="psR", bufs=1, space="PSUM") as psR:
                rl_ps = [psR.tile([E, NC], f32, tag=f"rl_{c}", name=f"rl_{c}")
                         for c in range(SC)]
                # ---- Phase E: h1, x2, router ----
                with tc.tile_pool(name="bigH1", bufs=1) as bigH1, \
                     tc.tile_pool(name="phE", bufs=3) as phE, \
                     tc.tile_pool(name="psE", bufs=1, space="PSUM") as psE, \
                     tc.tile_pool(name="psBC2", bufs=1, space="PSUM") as psBC2:
                    h1T = bigH1.tile([P, HK, S], f32)
                    ss2 = [psE.tile([1, NC], f32, tag=f"ss2_{c}", name=f"ss2_{c}")
                           for c in range(SC)]
                    for k in range(HK):
                        nc.sync.dma_start(h1T[:, k, :],
                                          ar1_out[k * P:(k + 1) * P, :])
                        sq = phE.tile([P, S], f32r, tag="sqe", name="sqe")
                        nc.vector.tensor_tensor(sq[:], h1T[:, k, :], h1T[:, k, :],
                                                ALU.mult)
                        sumsq_accum(ss2, sq, k, HK - 1)
                    rstd2 = rowstats(ss2, "n2")
                    bc2 = bcast_row(rstd2, bigE, psBC2, "n2")
                    ss3 = [psE.tile([1, NC], f32, tag=f"ss3_{c}", name=f"ss3_{c}")
                           for c in range(SC)]
                    for k in range(HK):
                        nc.vector.tensor_tensor(h1T[:, k, :], h1T[:, k, :],
                                                bc2[:], ALU.mult)
                        nc.vector.tensor_scalar_mul(h1T[:, k, :], h1T[:, k, :],
                                                    n2_sb[:, k:k + 1])
                        hs2 = phE.tile([P, S], f32, tag="hs2", name="hs2")
                        nc.sync.dma_start(hs2[:], hsT[k * P:(k + 1) * P, :])
                        nc.vector.tensor_tensor(h1T[:, k, :], h1T[:, k, :],
                                                hs2[:], ALU.add)
                        nc.sync.dma_start(h1_spill[k * P:(k + 1) * P, :],
                                          h1T[:, k, :])
                        sq = phE.tile([P, S], f32r, tag="sqe", name="sqe2")
                        nc.vector.tensor_tensor(sq[:], h1T[:, k, :], h1T[:, k, :],
                                                ALU.mult)
                        sumsq_accum(ss3, sq, k, HK - 1)
                    rstd3 = rowstats(ss3, "n3")
                    bc3 = bcast_row(rstd3, bigE, psBC2, "n3")
                    rt_sb = phE.tile([P, HK, E], f32, tag="rt", name="rt")
                    nc.sync.dma_start(rt_sb[:], router[:])
                    for k in range(HK):
                        x2f = phE.tile([P, S], f32, tag="x2f", name="x2f")
                        nc.vector.tensor_tensor(x2f[:], h1T[:, k, :], bc3[:],
                                                ALU.mult)
                        nc.vector.tensor_copy(x2bf[:, k, :], x2f[:])
                        for c in range(SC):
                            nc.tensor.matmul(rl_ps[c][:], rt_sb[:, k, :],
                                             x2f[:, c * NC:(c + 1) * NC],
                                             start=(k == 0), stop=(k == HK - 1))

                # ---- Phase F: gates (token-major: tokens on partitions) ----
                with tc.tile_pool(name="phF", bufs=1) as phF, \
                     tc.tile_pool(name="psF", bufs=1, space="PSUM") as psF:
                    rl_sb = phF.tile([E, S], f32, tag="rl", name="rl_sb")
                    for c in range(SC):
                        nc.vector.tensor_copy(rl_sb[:, c * NC:(c + 1) * NC],
                                              rl_ps[c][:])
                    nc.sync.dma_start(rl_out[:], rl_sb[:])
                    gate_row = phF.tile([1, S], f32, tag="grow", name="grow")
                    ohps = psF.tile([P, E], f32, tag="ohps", name="ohps")
                    nc.tensor.matmul(ohps[:], ones_m[:], onehot_sbr[:],
                                     start=True, stop=True)
                    ohbc = phF.tile([P, E], f32, tag="ohbc", name="ohbc")
                    nc.vector.tensor_copy(ohbc[:], ohps[:])
                    for j in range(SK):
                        ptr = psF.tile([P, E], f32, tag="ptrF", name="ptrF")
                        nc.tensor.transpose(ptr[:], rl_sb[:, j * P:(j + 1) * P],
                                            ident_sb[:E, :E])
                        rlN = phF.tile([P, E], f32, tag="rlN", name="rlN")
                        nc.vector.tensor_copy(rlN[:], ptr[:])
                        eu = phF.tile([P, E], f32, tag="euN", name="euN")
                        nc.scalar.activation(eu[:], rlN[:], AF.Exp)
                        m1 = phF.tile([P, 1], f32, tag="m1N", name="m1N")
                        nc.vector.tensor_reduce(m1[:], eu[:],
                                                mybir.AxisListType.X, ALU.max)
                        neq1 = phF.tile([P, E], f32, tag="neq1", name="neq1")
                        nc.vector.tensor_scalar(neq1[:], eu[:], m1[:], None,
                                                ALU.is_equal)
                        nc.vector.tensor_scalar(neq1[:], neq1[:], -1.0, 1.0,
                                                ALU.mult, ALU.add)
                        em = phF.tile([P, E], f32, tag="emN", name="emN")
                        nc.vector.tensor_tensor(em[:], eu[:], neq1[:], ALU.mult)
                        m2 = phF.tile([P, 1], f32, tag="m2N", name="m2N")
                        nc.vector.tensor_reduce(m2[:], em[:],
                                                mybir.AxisListType.X, ALU.max)
                        den = phF.tile([P, 1], f32, tag="denN", name="denN")
                        nc.vector.tensor_tensor(den[:], m1[:], m2[:], ALU.add)
                        rden = phF.tile([P, 1], f32, tag="rdenN", name="rdenN")
                        nc.vector.reciprocal(rden[:], den[:])
                        eisel = phF.tile([P, E], f32, tag="eisel", name="eisel")
                        nc.vector.tensor_tensor(eisel[:], eu[:], ohbc[:],
                                                ALU.mult)
                        eit = phF.tile([P, 1], f32, tag="eiN", name="eiN")
                        nc.vector.tensor_reduce(eit[:], eisel[:],
                                                mybir.AxisListType.X, ALU.add)
                        ei = eit[:]
                        s1 = phF.tile([P, 1], f32, tag="s1N", name="s1N")
                        s2 = phF.tile([P, 1], f32, tag="s2N", name="s2N")
                        nc.vector.tensor_tensor(s1[:], ei, m1[:], ALU.is_equal)
                        nc.vector.tensor_tensor(s2[:], ei, m2[:], ALU.is_equal)
                        nc.vector.tensor_tensor(s1[:], s1[:], s2[:], ALU.max)
                        g = phF.tile([P, 1], f32, tag="gN", name="gN")
                        nc.vector.tensor_tensor(g[:], ei, s1[:], ALU.mult)
                        nc.vector.tensor_tensor(g[:], g[:], rden[:], ALU.mult)
                        ptg = psF.tile([1, P], f32, tag="ptg", name="ptg")
                        nc.tensor.transpose(ptg[:], g[:], ident_sb[:])
                        nc.vector.tensor_copy(gate_row[:, j * P:(j + 1) * P],
                                              ptg[:])
                    for c in range(SC):
                        pf2 = psF.tile([P, NC], f32, tag="pf2", name="pfg")
                        nc.tensor.matmul(pf2[:], ones_m[:],
                                         gate_row[:, c * NC:(c + 1) * NC],
                                         start=True, stop=True)
                        nc.vector.tensor_copy(gate_bc[:, c * NC:(c + 1) * NC],
                                              pf2[:])

            # ---- Phase G: expert FFN (bf16) ----
            with tc.tile_pool(name="bigG", bufs=1) as bigG, \
                 tc.tile_pool(name="phG", bufs=2) as phG, \
                 tc.tile_pool(name="psG1", bufs=2, space="PSUM") as psG1, \
                 tc.tile_pool(name="psG2", bufs=2, space="PSUM") as psG2:
                hT = bigG.tile([P, FK, S], bf16)
                for mf in range(FK):
                    w1t = phG.tile([P, HK * P], bf16, tag="w1t", name="w1t")
                    nc.sync.dma_start(w1t[:], w1[mf])
                    vt = phG.tile([P, HK * P], bf16, tag="vt", name="vt")
                    nc.sync.dma_start(vt[:], wev[mf])
                    for c in range(SC):
                        pw = psG1.tile([P, NC], f32, tag="pw", name="pw")
                        pv = psG1.tile([P, NC], f32, tag="pvm", name="pvm")
                        for k in range(HK):
                            nc.tensor.matmul(pw[:], w1t[:, k * P:(k + 1) * P],
                                             x2bf[:, k, c * NC:(c + 1) * NC],
                                             start=(k == 0), stop=(k == HK - 1))
                        for k in range(HK):
                            nc.tensor.matmul(pv[:], vt[:, k * P:(k + 1) * P],
                                             x2bf[:, k, c * NC:(c + 1) * NC],
                                             start=(k == 0), stop=(k == HK - 1))
                        gl = phG.tile([P, NC], bf16, tag="gl", name="gl")
                        nc.scalar.activation(gl[:], pw[:], AF.Gelu)
                        nc.vector.tensor_tensor(hT[:, mf, c * NC:(c + 1) * NC],
                                                gl[:], pv[:], ALU.mult)
                for mo in range(HK):
                    dt_sb = phG.tile([P, FK * P], bf16, tag="dt", name="dt_sb")
                    nc.sync.dma_start(dt_sb[:], wed[mo])
                    stage = phG.tile([P, S], f32, tag="gstage", name="gstage")
                    for c in range(SC):
                        pd = psG2.tile([P, NC], f32, tag="pd", name="pd")
                        for k in range(FK):
                            nc.tensor.matmul(pd[:], dt_sb[:, k * P:(k + 1) * P],
                                             hT[:, k, c * NC:(c + 1) * NC],
                                             start=(k == 0), stop=(k == FK - 1))
                        nc.vector.tensor_tensor(
                            stage[:, c * NC:(c + 1) * NC], pd[:],
                            gate_bc[:, c * NC:(c + 1) * NC], ALU.mult)
                    nc.sync.dma_start(ar2_in[mo * P:(mo + 1) * P, :], stage[:])

        nc.gpsimd.collective_compute(
            "AllReduce", ALU.add, replica_groups=rg,
            ins=[ar2_in[0:_half, :]], outs=[ar2_out[0:_half, :]])
        nc.gpsimd.collective_compute(
            "AllReduce", ALU.add, replica_groups=rg,
            ins=[ar2_in[_half:H, :]], outs=[ar2_out[_half:H, :]])

        # ---- Phase H: final norm + residual ----
        with tc.tile_pool(name="bigM", bufs=1) as bigM, \
             tc.tile_pool(name="phH", bufs=3) as phH, \
             tc.tile_pool(name="psH", bufs=1, space="PSUM") as psH, \
             tc.tile_pool(name="psBC4", bufs=1, space="PSUM") as psBC4:
            moT = bigM.tile([P, HK, S], f32)
            ss4 = [psH.tile([1, NC], f32, tag=f"ss4_{c}", name=f"ss4_{c}")
                   for c in range(SC)]
            for k in range(HK):
                nc.sync.dma_start(moT[:, k, :], ar2_out[k * P:(k + 1) * P, :])
                sq = phH.tile([P, S], f32r, tag="sqh", name="sqh")
                nc.vector.tensor_tensor(sq[:], moT[:, k, :], moT[:, k, :],
                                        ALU.mult)
                sumsq_accum(ss4, sq, k, HK - 1)
            rstd4 = rowstats(ss4, "n4")
            bc4 = bcast_row(rstd4, bigM, psBC4, "n4")
            for k in range(HK):
                h1r = phH.tile([P, S], f32, tag="h1r", name="h1r")
                nc.sync.dma_start(h1r[:], h1_spill[k * P:(k + 1) * P, :])
                nc.vector.tensor_tensor(moT[:, k, :], moT[:, k, :], bc4[:],
                                        ALU.mult)
                nc.vector.tensor_scalar_mul(moT[:, k, :], moT[:, k, :],
                                            n4_sb[:, k:k + 1])
                ostage = phH.tile([P, S], f32, tag="ostage", name="ostage")
                nc.vector.tensor_tensor(ostage[:], moT[:, k, :], h1r[:], ALU.add)
                nc.sync.dma_start(outT[k * P:(k + 1) * P, :], ostage[:])

        const.release()
        rows.release()
    return nc


def host_prep(inputs_np, core, S, H, Dh, NQH, F):
    """Build the per-core in_map from full inputs (numpy)."""
    import numpy as np
    import ml_dtypes
    HK, FK = H // P, F // P
    DCOL = NQH * Dh
    DK = max(1, DCOL // P)
    E = 8
    f32 = np.float32
    i = core
    hs = np.ascontiguousarray(inputs_np["hidden_states"][0].T).astype(f32)
    n1 = inputs_np["n1"].astype(f32)
    n3 = inputs_np["n3"].astype(f32)
    wq_f = (n1[:, None] * inputs_np["wq"])[:, DCOL * i:DCOL * (i + 1)]
    wk_f = (n1[:, None] * inputs_np["wk"])[:, Dh * i:Dh * (i + 1)]
    wv_f = (n1[:, None] * inputs_np["wv"])[:, Dh * i:Dh * (i + 1)]
    wo_i = inputs_np["wo"][DCOL * i:DCOL * (i + 1), :]
    router_f = n3[:, None] * inputs_np["router"]
    w1_f = (n3[:, None] * inputs_np["we_w1"][i]).astype(ml_dtypes.bfloat16)
    wv_e = (n3[:, None] * inputs_np["we_v"][i]).astype(ml_dtypes.bfloat16)
    wd_e = inputs_np["we_d"][i].astype(ml_dtypes.bfloat16)

    def ktile(w, width):  # [H, width] -> [128, HK, width]
        return np.ascontiguousarray(
            w.reshape(HK, P, width).transpose(1, 0, 2)).astype(f32)

    inv = 1.0 / (10000.0 ** (np.arange(0, Dh, 2) / Dh))
    t = np.arange(S, dtype=np.float64)
    ph = t[:, None] * inv[None, :]
    ph = np.concatenate([ph, ph], -1)
    cosT = np.cos(ph).T.astype(f32)
    sinT = np.sin(ph).T.astype(f32)
    reps = P // Dh
    tril = np.tril(np.ones((P, P), f32))
    onehot_r = np.zeros((1, E), f32)
    onehot_r[0, i] = 1.0
    return {
        "hsT": hs,
        "wq": ktile(wq_f, DCOL),
        "wk": ktile(wk_f, Dh),
        "wv": ktile(wv_f, Dh),
        "wo": np.ascontiguousarray(
            wo_i.reshape(DK, P, H).transpose(1, 0, 2)).astype(f32),
        "router": ktile(router_f, E),
        "n2": np.ascontiguousarray(
            inputs_np["n2"].astype(f32).reshape(HK, P).T),
        "n4": np.ascontiguousarray(
            inputs_np["n4"].astype(f32).reshape(HK, P).T),
        "cos2": np.tile(cosT, (reps, 1)),
        "sin2": np.tile(sinT, (reps, 1)),
        "mtril": tril,
        "madd": np.where(tril > 0, 0.0, -50.0).astype(f32),
        "ident": np.eye(P, dtype=f32),
        "onehot_r": onehot_r,
        "w1": np.ascontiguousarray(
            w1_f.reshape(HK, P, FK, P).transpose(2, 1, 0, 3).reshape(FK, P, HK * P)),
        "wev": np.ascontiguousarray(
            wv_e.reshape(HK, P, FK, P).transpose(2, 1, 0, 3).reshape(FK, P, HK * P)),
        "wed": np.ascontiguousarray(
            wd_e.reshape(FK, P, HK, P).transpose(2, 1, 0, 3).reshape(HK, P, FK * P)),
    }


_CACHE = {}

S, H, Dh, NQH, F, NCORES = 1024, 2048, 64, 4, 4096, 8


def _get_nc():
    if "nc" not in _CACHE:
        from concourse import bacc
        nc = bacc.Bacc("TRN2", target_bir_lowering=False, debug=False,
                       num_devices=NCORES)
        build(nc, S, H, Dh, NQH, F, n_cores=NCORES)
        nc.compile()
        _CACHE["nc"] = nc
    return _CACHE["nc"]


def kernel(**inputs):
    from concourse.bass_utils import run_bass_kernel_spmd
    nc = _get_nc()
    inputs = {k: np.asarray(v) for k, v in inputs.items()}
    in_maps = [host_prep(inputs, i, S, H, Dh, NQH, F) for i in range(NCORES)]
    res = run_bass_kernel_spmd(nc, in_maps, list(range(NCORES))).results
    out = np.ascontiguousarray(res[0]["outT"].T)[None].astype(np.float32)
    rl = np.ascontiguousarray(res[0]["rl"].T).astype(np.float32)
    return out, rl
